# revision 3
# baseline (speedup 1.0000x reference)
"""DTW similarity kernel for Trainium2 (8 NeuronCores, SPMD bass/tile).

Per core (replicated; inputs identical on all cores):
  1. L2-normalize trajectory rows -> bf16 (DRAM bounce).
  2. DMA-transpose loads -> [D, N] bf16 operands in SBUF.
  3. cost = 1 - t1n @ t2n.T via PE matmuls -> f32 cost matrix in DRAM.
  4. DTW DP, skewed wavefront: strips of 128 rows on partitions, block
     width C. Cross-partition "up row" comes from a PE shift-matrix
     matmul into PSUM (SH1: out[p]=in[p-1]; SH2 injects the previous
     strip's last row into partition 0). ScalarE copies PSUM->SBUF,
     then VectorE: a = min(up, updiag); tensor_tensor_scan computes
     D_j = c_j + min(a_j, D_{j-1}) along the block.
  5. similarity = 1/(1+distance) -> scalar output.
"""

import sys

sys.path.insert(0, "/opt/trn_rl_repo")

import numpy as np  # noqa: E402

BIG = 1e30
NCORES = 8


def _build(N1, N2, D, C):
    from concourse import bacc
    import concourse.bass as bass
    import concourse.mybir as mybir
    import concourse.tile as tile

    f32 = mybir.dt.float32
    bf16 = mybir.dt.bfloat16
    P = 128
    assert N1 % P == 0 and N2 % C == 0 and D % P == 0 and N2 % 512 == 0
    nstrips = N1 // P
    B = N2 // C
    W = N2 + (P - 1) * C + 1
    KT = D // P
    NT = N2 // 512
    MT = N1 // P

    nc = bacc.Bacc(None, target_bir_lowering=False, debug=True, num_devices=NCORES)
    t1_ext = nc.dram_tensor("trajectory1", [N1, D], f32, kind="ExternalInput")
    t2_ext = nc.dram_tensor("trajectory2", [N2, D], f32, kind="ExternalInput")
    out_ext = nc.dram_tensor("out", [1, 1], f32, kind="ExternalOutput")

    mn = mybir.AluOpType.min
    ad = mybir.AluOpType.add
    ml = mybir.AluOpType.mult
    eq = mybir.AluOpType.is_equal
    AF = mybir.ActivationFunctionType

    with tile.TileContext(nc) as tc:
        with tc.tile_pool(name="dram", bufs=1, space="DRAM") as dram:
            t1n_dram = dram.tile([N1, D], bf16)
            t2n_dram = dram.tile([N2, D], bf16)
            cost_dram = dram.tile([N1, N2], f32)

            # ---- phase A: normalize rows, write bf16 bounce ----
            with tc.tile_pool(name="norm", bufs=3) as pn, \
                 tc.tile_pool(name="norms", bufs=4) as ps:
                for src, dst, n in ((t1_ext, t1n_dram, N1), (t2_ext, t2n_dram, N2)):
                    for i in range(n // P):
                        x = pn.tile([P, D], f32, tag="x")
                        nc.sync.dma_start(out=x[:], in_=src[i * P:(i + 1) * P, :])
                        sq = pn.tile([P, D], f32, tag="sq")
                        ss = ps.tile([P, 1], f32, tag="ss")
                        nc.scalar.activation(
                            out=sq[:], in_=x[:], func=AF.Square, accum_out=ss[:])
                        nc.scalar.activation(out=ss[:], in_=ss[:], func=AF.Sqrt)
                        nc.vector.tensor_scalar(ss[:], ss[:], 1e-8, None, ad)
                        r = ps.tile([P, 1], f32, tag="r")
                        nc.vector.reciprocal(r[:], ss[:])
                        y = pn.tile([P, D], bf16, tag="y")
                        nc.vector.tensor_tensor(
                            y[:], x[:], r[:].to_broadcast((P, D)), ml)
                        nc.sync.dma_start(out=dst[i * P:(i + 1) * P, :], in_=y[:])

            # ---- phase B+C: transposed loads + GEMM ----
            with tc.tile_pool(name="ops", bufs=1) as pg, \
                 tc.tile_pool(name="psum", bufs=4, space="PSUM") as pp, \
                 tc.tile_pool(name="bounce", bufs=4) as pb:
                t1T = []
                t2T = []
                for k in range(KT):
                    a = pg.tile([P, N1], bf16, tag=f"t1T{k}")
                    nc.sync.dma_start_transpose(a[:], t1n_dram[:, k * P:(k + 1) * P])
                    t1T.append(a)
                for k in range(KT):
                    a = pg.tile([P, N2], bf16, tag=f"t2T{k}")
                    nc.sync.dma_start_transpose(a[:], t2n_dram[:, k * P:(k + 1) * P])
                    t2T.append(a)
                for m in range(MT):
                    for n in range(NT):
                        acc = pp.tile([P, 512], f32, tag="acc")
                        for k in range(KT):
                            nc.tensor.matmul(
                                acc[:],
                                t1T[k][:, m * P:(m + 1) * P],
                                t2T[k][:, n * 512:(n + 1) * 512],
                                start=(k == 0), stop=(k == KT - 1))
                        b = pb.tile([P, 512], f32, tag="b")
                        nc.vector.tensor_scalar(b[:], acc[:], -1.0, 1.0, ml, ad)
                        nc.sync.dma_start(
                            out=cost_dram[m * P:(m + 1) * P, n * 512:(n + 1) * 512],
                            in_=b[:])

            # ---- phase D: DTW DP ----
            with tc.tile_pool(name="dconst", bufs=1) as pc, \
                 tc.tile_pool(name="dcs", bufs=2) as pcs, \
                 tc.tile_pool(name="dD", bufs=2) as pD, \
                 tc.tile_pool(name="dps", bufs=4, space="PSUM") as pps, \
                 tc.tile_pool(name="da", bufs=4) as pa:
                # constants: shift matrices, BIG tile, iotas
                iop = pc.tile([P, P], f32, tag="iop")   # value = partition idx
                iof = pc.tile([P, P], f32, tag="iof")   # value = free idx
                nc.gpsimd.iota(iof[:], [[1, P]], channel_multiplier=0, allow_small_or_imprecise_dtypes=True)
                nc.gpsimd.iota(iop[:], [[0, P]], channel_multiplier=1, allow_small_or_imprecise_dtypes=True)
                sh1 = pc.tile([P, P], f32, tag="sh1")   # sh1[k,m] = (k+1 == m)
                nc.vector.tensor_scalar(sh1[:], iop[:], 1.0, None, ad)
                nc.vector.tensor_tensor(sh1[:], sh1[:], iof[:], eq)
                sh2 = pc.tile([P, P], f32, tag="sh2")   # sh2[k,m] = (k==127)*(m==0)
                tmp = pc.tile([P, P], f32, tag="tmp")
                nc.vector.tensor_scalar(sh2[:], iop[:], float(P - 1), None, eq)
                nc.vector.tensor_scalar(tmp[:], iof[:], 0.0, None, eq)
                nc.vector.tensor_tensor(sh2[:], sh2[:], tmp[:], ml)
                bigt = pc.tile([P, C + 1], f32, tag="bigt")
                nc.vector.memset(bigt[:], BIG)

                cost_base = cost_dram[:, :]
                Dprev = None
                for s in range(nstrips):
                    cs = pcs.tile([P, W], f32, tag="cs")
                    src = bass.AP(
                        tensor=cost_base.tensor,
                        offset=cost_base.offset + s * P * N2,
                        ap=[[N2 - C, P], [1, W - 1]])
                    nc.sync.dma_start(out=cs[:, 1:W], in_=src)
                    Dt = pD.tile([P, W], f32, tag="Dt")
                    nc.gpsimd.memset(Dt[:], BIG)
                    for t in range(B + P - 1):
                        phi = min(t, P - 1)
                        na = phi + 1
                        tm = min(t, B - 1)  # clamp for in-bounds junk reads
                        ps_t = pps.tile([P, C + 1], f32, tag="shps")
                        if t > 0:
                            nc.tensor.matmul(
                                ps_t[0:na, :], sh1[:, 0:na],
                                Dt[:, (t - 1) * C:t * C + 1],
                                start=True, stop=False)
                        rhs2 = (bigt[:, 0:C + 1] if s == 0 else
                                Dprev[:, (tm + P - 1) * C:(tm + P - 1) * C + C + 1])
                        nc.tensor.matmul(
                            ps_t[0:na, :], sh2[:, 0:na], rhs2,
                            start=(t == 0), stop=True)
                        u = pa.tile([P, C + 1], f32, tag="u")
                        nc.scalar.activation(
                            out=u[0:na, :], in_=ps_t[0:na, :], func=AF.Copy)
                        a = pa.tile([P, C], f32, tag="a")
                        nc.vector.tensor_tensor(
                            a[0:na, :], u[0:na, 1:C + 1], u[0:na, 0:C], mn)
                        if s == 0 and t == 0:
                            nc.vector.memset(a[0:1, 0:1], 0.0)
                        xs = t * C + 1
                        nc.vector.tensor_tensor_scan(
                            Dt[0:na, xs:xs + C],
                            a[0:na, :],
                            cs[0:na, xs:xs + C],
                            Dt[0:na, xs - 1:xs],
                            mn, ad)
                    Dprev = Dt

                # similarity = 1/(1+distance); move off partition 127 via DMA
                r0 = pa.tile([1, 1], f32, tag="r0")
                nc.sync.dma_start(out=r0[:], in_=Dprev[P - 1:P, W - 1:W])
                nc.vector.tensor_scalar(r0[:], r0[:], 1.0, None, ad)
                nc.vector.reciprocal(r0[:], r0[:])
                nc.sync.dma_start(out=out_ext[:], in_=r0[:])

    nc.finalize()
    return nc


_cache = {}


def _get_nc(N1, N2, D, C):
    key = (N1, N2, D, C)
    if key not in _cache:
        _cache[key] = _build(N1, N2, D, C)
    return _cache[key]


def run(trajectory1, trajectory2, C=64, trace=False):
    from concourse.bass_utils import run_bass_kernel_spmd

    N1, D = trajectory1.shape
    N2, _ = trajectory2.shape
    nc = _get_nc(N1, N2, D, C)
    in_map = {
        "trajectory1": np.ascontiguousarray(trajectory1, dtype=np.float32),
        "trajectory2": np.ascontiguousarray(trajectory2, dtype=np.float32),
    }
    res = run_bass_kernel_spmd(
        nc, [in_map] * NCORES, list(range(NCORES)), trace=trace)
    out = res.results[0]["out"]
    return np.float32(out.reshape(())), res


def kernel(trajectory1, trajectory2):
    out, _ = run(trajectory1, trajectory2)
    return out


# revision 5
# speedup vs baseline: 10.7296x; 10.7296x over previous
"""DTW similarity kernel for Trainium2 (8 NeuronCores, SPMD bass/tile).

Per core (replicated; inputs identical on all cores):
  1. L2-normalize trajectory rows -> bf16 (DRAM bounce).
  2. DMA-transpose loads -> [D, N] bf16 operands in SBUF.
  3. cost = 1 - t1n @ t2n.T via PE matmuls -> f32 cost matrix in DRAM.
  4. DTW DP, skewed wavefront: strips of 128 rows on partitions, block
     width C. Cross-partition "up row" comes from a PE shift-matrix
     matmul into PSUM (SH1: out[p]=in[p-1]; SH2 injects the previous
     strip's last row into partition 0). ScalarE copies PSUM->SBUF,
     then VectorE: a = min(up, updiag); tensor_tensor_scan computes
     D_j = c_j + min(a_j, D_{j-1}) along the block.
  5. similarity = 1/(1+distance) -> scalar output.
"""

import sys

sys.path.insert(0, "/opt/trn_rl_repo")

import numpy as np  # noqa: E402

BIG = 1e30
NCORES = 8


def _build(N1, N2, D, C):
    from concourse import bacc
    import concourse.bass as bass
    import concourse.mybir as mybir
    import concourse.tile as tile

    f32 = mybir.dt.float32
    bf16 = mybir.dt.bfloat16
    P = 128
    assert N1 % P == 0 and N2 % C == 0 and D % P == 0 and N2 % 512 == 0
    nstrips = N1 // P
    B = N2 // C
    W = N2 + (P - 1) * C + 1
    KT = D // P
    NT = N2 // 512
    MT = N1 // P

    nc = bacc.Bacc(None, target_bir_lowering=False, debug=True, num_devices=NCORES)
    t1_ext = nc.dram_tensor("trajectory1", [N1, D], f32, kind="ExternalInput")
    t2_ext = nc.dram_tensor("trajectory2", [N2, D], f32, kind="ExternalInput")
    out_ext = nc.dram_tensor("out", [1, 1], f32, kind="ExternalOutput")

    mn = mybir.AluOpType.min
    ad = mybir.AluOpType.add
    ml = mybir.AluOpType.mult
    eq = mybir.AluOpType.is_equal
    AF = mybir.ActivationFunctionType

    with tile.TileContext(nc) as tc:
        with tc.tile_pool(name="dram", bufs=1, space="DRAM") as dram:
            t1n_dram = dram.tile([N1, D], bf16)
            t2n_dram = dram.tile([N2, D], bf16)
            cost_dram = dram.tile([N1, N2], f32)

            # ---- phase A: normalize rows, write bf16 bounce ----
            with tc.tile_pool(name="norm", bufs=3) as pn, \
                 tc.tile_pool(name="norms", bufs=4) as ps:
                for src, dst, n in ((t1_ext, t1n_dram, N1), (t2_ext, t2n_dram, N2)):
                    for i in range(n // P):
                        x = pn.tile([P, D], f32, tag="x")
                        nc.sync.dma_start(out=x[:], in_=src[i * P:(i + 1) * P, :])
                        sq = pn.tile([P, D], f32, tag="sq")
                        ss = ps.tile([P, 1], f32, tag="ss")
                        nc.scalar.activation(
                            out=sq[:], in_=x[:], func=AF.Square, accum_out=ss[:])
                        nc.scalar.activation(out=ss[:], in_=ss[:], func=AF.Sqrt)
                        nc.vector.tensor_scalar(ss[:], ss[:], 1e-8, None, ad)
                        r = ps.tile([P, 1], f32, tag="r")
                        nc.vector.reciprocal(r[:], ss[:])
                        y = pn.tile([P, D], bf16, tag="y")
                        nc.vector.tensor_tensor(
                            y[:], x[:], r[:].to_broadcast((P, D)), ml)
                        nc.sync.dma_start(out=dst[i * P:(i + 1) * P, :], in_=y[:])

            # ---- phase B+C: transposed loads + GEMM ----
            with tc.tile_pool(name="ops", bufs=1) as pg, \
                 tc.tile_pool(name="psum", bufs=4, space="PSUM") as pp, \
                 tc.tile_pool(name="bounce", bufs=4) as pb:
                t1T = []
                t2T = []
                for k in range(KT):
                    a = pg.tile([P, N1], bf16, tag=f"t1T{k}")
                    nc.sync.dma_start_transpose(a[:], t1n_dram[:, k * P:(k + 1) * P])
                    t1T.append(a)
                for k in range(KT):
                    a = pg.tile([P, N2], bf16, tag=f"t2T{k}")
                    nc.sync.dma_start_transpose(a[:], t2n_dram[:, k * P:(k + 1) * P])
                    t2T.append(a)
                for m in range(MT):
                    for n in range(NT):
                        acc = pp.tile([P, 512], f32, tag="acc")
                        for k in range(KT):
                            nc.tensor.matmul(
                                acc[:],
                                t1T[k][:, m * P:(m + 1) * P],
                                t2T[k][:, n * 512:(n + 1) * 512],
                                start=(k == 0), stop=(k == KT - 1))
                        b = pb.tile([P, 512], f32, tag="b")
                        nc.vector.tensor_scalar(b[:], acc[:], -1.0, 1.0, ml, ad)
                        nc.sync.dma_start(
                            out=cost_dram[m * P:(m + 1) * P, n * 512:(n + 1) * 512],
                            in_=b[:])

            # ---- phase D: DTW DP ----
            with tc.tile_pool(name="dconst", bufs=1) as pc, \
                 tc.tile_pool(name="dcs", bufs=2) as pcs, \
                 tc.tile_pool(name="dD", bufs=2) as pD, \
                 tc.tile_pool(name="dps", bufs=6, space="PSUM") as pps, \
                 tc.tile_pool(name="da", bufs=8) as pa:
                # constants: shift matrices, BIG tile, iotas
                iop = pc.tile([P, P], f32, tag="iop")   # value = partition idx
                iof = pc.tile([P, P], f32, tag="iof")   # value = free idx
                nc.gpsimd.iota(iof[:], [[1, P]], channel_multiplier=0, allow_small_or_imprecise_dtypes=True)
                nc.gpsimd.iota(iop[:], [[0, P]], channel_multiplier=1, allow_small_or_imprecise_dtypes=True)
                sh1 = pc.tile([P, P], f32, tag="sh1")   # sh1[k,m] = (k+1 == m)
                nc.vector.tensor_scalar(sh1[:], iop[:], 1.0, None, ad)
                nc.vector.tensor_tensor(sh1[:], sh1[:], iof[:], eq)
                sh2 = pc.tile([P, P], f32, tag="sh2")   # sh2[k,m] = (k==127)*(m==0)
                tmp = pc.tile([P, P], f32, tag="tmp")
                nc.vector.tensor_scalar(sh2[:], iop[:], float(P - 1), None, eq)
                nc.vector.tensor_scalar(tmp[:], iof[:], 0.0, None, eq)
                nc.vector.tensor_tensor(sh2[:], sh2[:], tmp[:], ml)
                bigt = pc.tile([P, C + 1], f32, tag="bigt")
                nc.vector.memset(bigt[:], BIG)

                cost_base = cost_dram[:, :]
                Dprev = None
                for s in range(nstrips):
                    cs = pcs.tile([P, W], f32, tag="cs")
                    src = bass.AP(
                        tensor=cost_base.tensor,
                        offset=cost_base.offset + s * P * N2,
                        ap=[[N2 - C, P], [1, W - 1]])
                    nc.sync.dma_start(out=cs[:, 1:W], in_=src)
                    Dt = pD.tile([P, W], f32, tag="Dt")
                    nc.gpsimd.memset(Dt[:], BIG)
                    for t in range(B + P - 1):
                        phi = min(t, P - 1)
                        na = phi + 1
                        tm = min(t, B - 1)  # clamp for in-bounds junk reads
                        ps_t = pps.tile([P, C + 1], f32, tag="shps")
                        if t > 0:
                            nc.tensor.matmul(
                                ps_t[0:na, :], sh1[:, 0:na],
                                Dt[:, (t - 1) * C:t * C + 1],
                                start=True, stop=False)
                        rhs2 = (bigt[:, 0:C + 1] if s == 0 else
                                Dprev[:, (tm + P - 1) * C:(tm + P - 1) * C + C + 1])
                        nc.tensor.matmul(
                            ps_t[0:na, :], sh2[:, 0:na], rhs2,
                            start=(t == 0), stop=True)
                        u = pa.tile([P, C + 1], f32, tag="u")
                        nc.vector.tensor_copy(u[0:na, :], ps_t[0:na, :])
                        a = pa.tile([P, C], f32, tag="a")
                        nc.vector.tensor_tensor(
                            a[0:na, :], u[0:na, 1:C + 1], u[0:na, 0:C], mn)
                        if s == 0 and t == 0:
                            nc.vector.memset(a[0:1, 0:1], 0.0)
                        xs = t * C + 1
                        nc.vector.tensor_tensor_scan(
                            Dt[0:na, xs:xs + C],
                            a[0:na, :],
                            cs[0:na, xs:xs + C],
                            Dt[0:na, xs - 1:xs],
                            mn, ad)
                    Dprev = Dt

                # similarity = 1/(1+distance); move off partition 127 via DMA
                r0 = pa.tile([1, 1], f32, tag="r0")
                nc.sync.dma_start(out=r0[:], in_=Dprev[P - 1:P, W - 1:W])
                nc.vector.tensor_scalar(r0[:], r0[:], 1.0, None, ad)
                nc.vector.reciprocal(r0[:], r0[:])
                nc.sync.dma_start(out=out_ext[:], in_=r0[:])

    nc.finalize()
    return nc


_cache = {}


def _get_nc(N1, N2, D, C):
    key = (N1, N2, D, C)
    if key not in _cache:
        _cache[key] = _build(N1, N2, D, C)
    return _cache[key]


def run(trajectory1, trajectory2, C=64, trace=False):
    from concourse.bass_utils import run_bass_kernel_spmd

    N1, D = trajectory1.shape
    N2, _ = trajectory2.shape
    nc = _get_nc(N1, N2, D, C)
    in_map = {
        "trajectory1": np.ascontiguousarray(trajectory1, dtype=np.float32),
        "trajectory2": np.ascontiguousarray(trajectory2, dtype=np.float32),
    }
    res = run_bass_kernel_spmd(
        nc, [in_map] * NCORES, list(range(NCORES)), trace=trace)
    out = res.results[0]["out"]
    return np.float32(out.reshape(())), res


def kernel(trajectory1, trajectory2):
    out, _ = run(trajectory1, trajectory2)
    return out
